# revision 1
# baseline (speedup 1.0000x reference)
"""Trainium2 Bass kernel for batched DTW (nn_DTW_82085414961776).

reference:
  a_p = a @ W1.T + b1                      # [B, L1]
  b_p = b @ W2.T + b2                      # [B, L2]
  cost[b,i,j] = (a_p[b,i] - b_p[b,j])^2
  out[b] = DTW(cost[b])   with R[i,j] = c + min(R[i-1,j-1], R[i-1,j], R[i,j-1])

Sharding: data-parallel over batch. 8 cores x 8 sequences each; weights
replicated. Each core runs the full [1024 x 1024] DP for its 8 sequences,
with the 8 sequences living on 8 SBUF partitions.

Row recurrence mapped to hardware:
  m[j] = min(x_prev[j], x_prev[j-1])       -> one scalar_tensor_tensor (min,min)
  x[j] = min(m[j], x[j-1]) + c[j]          -> one tensor_tensor_scan (op0=min, op1=add)
Cost rows are produced on the scalar (ACT) engine one row ahead:
  c_i = Square(b_p * 1 + (-a_p[:, i]))     (bias is a per-partition scalar)
The two linear projections run on the PE once at the start (weights are
pre-transposed on the host so lhsT/rhs layouts are direct loads).
"""

import sys

for _p in ("/opt/trn_rl_repo", "/opt/trn_rl_repo/concourse"):
    if _p not in sys.path:
        sys.path.insert(0, _p)

import numpy as np
from contextlib import ExitStack

import concourse.bass as bass
import concourse.tile as tile
import concourse.mybir as mybir
from concourse import bass_utils, bacc

BIG = 3.0e38
N_CORES = 8
B_FULL, L1, L2 = 64, 1024, 1024
BC = B_FULL // N_CORES  # batches per core


def build_dtw_module(bc: int, l1: int, l2: int):
    """Build the per-core Bass module. Inputs (per core):
       aT   [l1, bc]   - a slice, transposed
       bT   [l2, bc]   - b slice, transposed
       w1t  [l1, l1]   - W1.T (contraction dim on rows)
       w2t  [l2, l2]   - W2.T
       b1r  [1, l1]
       b2r  [1, l2]
     Output: out [bc, 1] DTW distances.
    """
    assert l1 % 128 == 0 and l2 % 128 == 0
    nkb1, nkb2 = l1 // 128, l2 // 128
    NBLK = min(512, l1, l2)
    dt = mybir.dt.float32
    Alu = mybir.AluOpType
    Act = mybir.ActivationFunctionType

    nc = bacc.Bacc("TRN2", target_bir_lowering=False, debug=False,
                   enable_asserts=False, num_devices=1)

    aT_d = nc.dram_tensor("aT", [l1, bc], dt, kind="ExternalInput").ap()
    bT_d = nc.dram_tensor("bT", [l2, bc], dt, kind="ExternalInput").ap()
    w1t_d = nc.dram_tensor("w1t", [l1, l1], dt, kind="ExternalInput").ap()
    w2t_d = nc.dram_tensor("w2t", [l2, l2], dt, kind="ExternalInput").ap()
    b1_d = nc.dram_tensor("b1r", [1, l1], dt, kind="ExternalInput").ap()
    b2_d = nc.dram_tensor("b2r", [1, l2], dt, kind="ExternalInput").ap()
    out_d = nc.dram_tensor("out", [bc, 1], dt, kind="ExternalOutput").ap()

    with tile.TileContext(nc) as tc, ExitStack() as ctx:
        pool = ctx.enter_context(tc.tile_pool(name="pool", bufs=1))
        wpool = ctx.enter_context(tc.tile_pool(name="wpool", bufs=1))
        cpool = ctx.enter_context(tc.tile_pool(name="cpool", bufs=3))
        psum = ctx.enter_context(tc.tile_pool(name="psum", bufs=2, space="PSUM"))

        # ---- load inputs ----
        # aT as [128, nkb1*bc]: column block kb holds aT[kb*128:(kb+1)*128, :]
        aT_sb = pool.tile([128, nkb1 * bc], dt)
        for kb in range(nkb1):
            nc.sync.dma_start(aT_sb[:, kb * bc:(kb + 1) * bc],
                              aT_d[kb * 128:(kb + 1) * 128, :])
        bT_sb = pool.tile([128, nkb2 * bc], dt)
        for kb in range(nkb2):
            nc.sync.dma_start(bT_sb[:, kb * bc:(kb + 1) * bc],
                              bT_d[kb * 128:(kb + 1) * 128, :])
        b1_sb = pool.tile([1, l1], dt)
        nc.sync.dma_start(b1_sb[:], b1_d[:])
        b2_sb = pool.tile([1, l2], dt)
        nc.sync.dma_start(b2_sb[:], b2_d[:])

        w1t_sb = [wpool.tile([128, l1], dt, name=f"w1t_sb{kb}")
                  for kb in range(nkb1)]
        for kb in range(nkb1):
            nc.sync.dma_start(w1t_sb[kb][:], w1t_d[kb * 128:(kb + 1) * 128, :])
        w2t_sb = [wpool.tile([128, l2], dt, name=f"w2t_sb{kb}")
                  for kb in range(nkb2)]
        for kb in range(nkb2):
            nc.sync.dma_start(w2t_sb[kb][:], w2t_d[kb * 128:(kb + 1) * 128, :])

        ones = pool.tile([1, bc], dt)
        nc.vector.memset(ones[:], 1.0)

        # ---- projections on PE ----
        # neg_a_p = -(a @ W1.T + b1) : [bc, l1]
        neg_a_p = pool.tile([bc, l1], dt)
        b_p = pool.tile([bc, l2], dt)
        for (dst, src_sb, w_sb, bias_sb, l, nkb, negate) in (
                (neg_a_p, aT_sb, w1t_sb, b1_sb, l1, nkb1, True),
                (b_p, bT_sb, w2t_sb, b2_sb, l2, nkb2, False)):
            for nb in range(l // NBLK):
                acc = psum.tile([bc, NBLK], dt, tag="mmacc")
                for kb in range(nkb):
                    nc.tensor.matmul(
                        acc[:],
                        src_sb[:, kb * bc:(kb + 1) * bc],
                        w_sb[kb][:, nb * NBLK:(nb + 1) * NBLK],
                        start=(kb == 0), stop=False)
                nc.tensor.matmul(
                    acc[:], ones[:], bias_sb[:, nb * NBLK:(nb + 1) * NBLK],
                    start=False, stop=True)
                if negate:
                    nc.scalar.mul(dst[:, nb * NBLK:(nb + 1) * NBLK], acc[:], -1.0)
                else:
                    nc.scalar.copy(dst[:, nb * NBLK:(nb + 1) * NBLK], acc[:])

        # ---- DTW DP ----
        x = pool.tile([bc, l2 + 1], dt)     # col0 = BIG left pad
        big = pool.tile([bc, l2], dt)
        m = pool.tile([bc, l2], dt)
        nc.vector.memset(big[:], BIG)
        nc.vector.memset(x[:, 0:1], BIG)

        for i in range(l1):
            c_i = cpool.tile([bc, l2], dt, tag="c")
            nc.scalar.activation(c_i[:], b_p[:], Act.Square,
                                 bias=neg_a_p[:, i:i + 1], scale=1.0)
            if i == 0:
                nc.vector.tensor_tensor_scan(
                    x[:, 1:l2 + 1], big[:], c_i[:], 0.0,
                    op0=Alu.min, op1=Alu.add)
            else:
                nc.vector.scalar_tensor_tensor(
                    m[:], x[:, 1:l2 + 1], BIG, x[:, 0:l2],
                    op0=Alu.min, op1=Alu.min)
                nc.vector.tensor_tensor_scan(
                    x[:, 1:l2 + 1], m[:], c_i[:], BIG,
                    op0=Alu.min, op1=Alu.add)

        nc.sync.dma_start(out_d[:], x[:, l2:l2 + 1])

    nc.compile()
    return nc


def _prep_in_maps(a, b, W1, b1, W2, b2, n_cores, bc):
    w1t = np.ascontiguousarray(W1.T)
    w2t = np.ascontiguousarray(W2.T)
    b1r = np.ascontiguousarray(b1.reshape(1, -1))
    b2r = np.ascontiguousarray(b2.reshape(1, -1))
    in_maps = []
    for c in range(n_cores):
        sl = slice(c * bc, (c + 1) * bc)
        in_maps.append({
            "aT": np.ascontiguousarray(a[sl].T),
            "bT": np.ascontiguousarray(b[sl].T),
            "w1t": w1t, "w2t": w2t, "b1r": b1r, "b2r": b2r,
        })
    return in_maps


_CACHE = {}


def _get_module():
    if "nc" not in _CACHE:
        _CACHE["nc"] = build_dtw_module(BC, L1, L2)
    return _CACHE["nc"]


def kernel(a, b, W1, b1, W2, b2, _trace=False):
    a = np.asarray(a, dtype=np.float32)
    b = np.asarray(b, dtype=np.float32)
    nc = _get_module()
    in_maps = _prep_in_maps(a, b, np.asarray(W1), np.asarray(b1),
                            np.asarray(W2), np.asarray(b2), N_CORES, BC)
    res = bass_utils.run_bass_kernel_spmd(
        nc, in_maps, core_ids=list(range(N_CORES)), trace=_trace)
    out = np.concatenate([res.results[c]["out"].reshape(BC)
                          for c in range(N_CORES)])
    if _trace:
        return out, res
    return out

